# revision 3
# baseline (speedup 1.0000x reference)
"""CoAttentionNetwork Trainium2 kernel — 8-core data parallel over batch.

Takes FULL inputs (B=64), shards batch across 8 NeuronCores (8 batches per
core), runs a Bass/Tile kernel per core, gathers per-core [8,6] outputs.
"""

import numpy as np

B, N, T, D, K, OUT = 64, 1024, 512, 384, 2, 6
N_CORES = 8
BPC = B // N_CORES  # batches per core
P = 128
NCH = N // P   # 8 n-chunks
TCH = T // P   # 4 t-chunks
DCH = D // P   # 3 d-chunks
FCH = 2 * D // P  # 6 chunks of concat dim

_BUILT = {}


def _build_nc():
    import concourse.bacc as bacc
    import concourse.mybir as mybir
    import concourse.tile as tile

    dt = mybir.dt.float32
    AF = mybir.ActivationFunctionType
    AX = mybir.AxisListType
    ALU = mybir.AluOpType

    nc = bacc.Bacc(None, target_bir_lowering=False, debug=False)

    S_d = nc.dram_tensor("S", [BPC, N, D], dt, kind="ExternalInput")
    ST_d = nc.dram_tensor("ST", [BPC, D, N], dt, kind="ExternalInput")
    C_d = nc.dram_tensor("C", [BPC, T, D], dt, kind="ExternalInput")
    CT_d = nc.dram_tensor("CT", [BPC, D, T], dt, kind="ExternalInput")
    Wl_d = nc.dram_tensor("Wl", [D, D], dt, kind="ExternalInput")
    WsT_d = nc.dram_tensor("WsT", [D, K], dt, kind="ExternalInput")
    WcT_d = nc.dram_tensor("WcT", [D, K], dt, kind="ExternalInput")
    whs_d = nc.dram_tensor("whs_bc", [P, K * NCH], dt, kind="ExternalInput")
    whc_d = nc.dram_tensor("whc_bc", [P, K * TCH], dt, kind="ExternalInput")
    fcwT_d = nc.dram_tensor("fcwT", [2 * D, OUT], dt, kind="ExternalInput")
    fcb_d = nc.dram_tensor("fcb", [1, OUT], dt, kind="ExternalInput")
    ones_d = nc.dram_tensor("ones", [P, 1], dt, kind="ExternalInput")
    out_d = nc.dram_tensor("out", [1, BPC * OUT], dt, kind="ExternalOutput")

    with tile.TileContext(nc) as tc:
        with (
            tc.tile_pool(name="wpool", bufs=1) as wpool,
            tc.tile_pool(name="io", bufs=2) as io,
            tc.tile_pool(name="work", bufs=2) as work,
            tc.tile_pool(name="pbig", bufs=4, space="PSUM") as pbig,
            tc.tile_pool(name="psmall", bufs=2, space="PSUM") as psmall,
            tc.tile_pool(name="pone", bufs=2, space="PSUM") as pone,
        ):
            # ---- constants / weights (loaded once) ----
            wl_sb = wpool.tile([P, DCH, D], dt)
            nc.sync.dma_start(wl_sb[:], Wl_d.rearrange("(c p) m -> p c m", p=P))
            wst_sb = wpool.tile([P, DCH, K], dt)
            nc.sync.dma_start(wst_sb[:], WsT_d.rearrange("(c p) k -> p c k", p=P))
            wct_sb = wpool.tile([P, DCH, K], dt)
            nc.sync.dma_start(wct_sb[:], WcT_d.rearrange("(c p) k -> p c k", p=P))
            whs_sb = wpool.tile([P, K * NCH], dt)
            nc.sync.dma_start(whs_sb[:], whs_d[:])
            whc_sb = wpool.tile([P, K * TCH], dt)
            nc.sync.dma_start(whc_sb[:], whc_d[:])
            fcw_sb = wpool.tile([P, FCH, OUT], dt)
            nc.sync.dma_start(fcw_sb[:], fcwT_d.rearrange("(c p) o -> p c o", p=P))
            fcb_sb = wpool.tile([1, OUT], dt)
            nc.sync.dma_start(fcb_sb[:], fcb_d[:])
            ones_sb = wpool.tile([P, 1], dt)
            nc.sync.dma_start(ones_sb[:], ones_d[:])
            out_sb = wpool.tile([1, BPC * OUT], dt)

            for b in range(BPC):
                # ---- input DMAs for this batch ----
                s_nat = io.tile([P, NCH, D], dt)
                nc.sync.dma_start(s_nat[:], S_d[b].rearrange("(c p) d -> p c d", p=P))
                st = io.tile([P, DCH, N], dt)
                nc.sync.dma_start(st[:], ST_d[b].rearrange("(c p) n -> p c n", p=P))
                c_nat = io.tile([P, TCH, D], dt)
                nc.sync.dma_start(c_nat[:], C_d[b].rearrange("(c p) d -> p c d", p=P))
                ct = io.tile([P, DCH, T], dt)
                nc.sync.dma_start(ct[:], CT_d[b].rearrange("(c p) t -> p c t", p=P))

                # ---- CWlT [D', t] = Wl^T @ C^T : 3 chunks of [128, 512] ----
                cwlt = work.tile([P, DCH, T], dt)
                for dc in range(DCH):
                    pb = pbig.tile([P, T], dt, tag="pbig")
                    for kd in range(DCH):
                        nc.tensor.matmul(
                            pb[:],
                            wl_sb[:, kd, dc * P:(dc + 1) * P],
                            ct[:, kd, :],
                            start=(kd == 0), stop=(kd == DCH - 1),
                        )
                    nc.vector.tensor_copy(cwlt[:, dc, :], pb[:])

                # ---- F [t, n] = tanh(CWlT^T @ ST): 4 t-chunks x 2 n-halves ----
                f_sb = work.tile([P, TCH, N], dt)
                for tcI in range(TCH):
                    for nh in range(2):
                        pb = pbig.tile([P, T], dt, tag="pbig")
                        for kd in range(DCH):
                            nc.tensor.matmul(
                                pb[:],
                                cwlt[:, kd, tcI * P:(tcI + 1) * P],
                                st[:, kd, nh * 512:(nh + 1) * 512],
                                start=(kd == 0), stop=(kd == DCH - 1),
                            )
                        nc.scalar.activation(
                            f_sb[:, tcI, nh * 512:(nh + 1) * 512], pb[:], AF.Tanh)

                # ---- FT [n, t] = tanh(ST^T @ CWlT): 8 n-chunks ----
                ft_sb = work.tile([P, NCH, T], dt)
                for ncI in range(NCH):
                    pb = pbig.tile([P, T], dt, tag="pbig")
                    for kd in range(DCH):
                        nc.tensor.matmul(
                            pb[:],
                            st[:, kd, ncI * P:(ncI + 1) * P],
                            cwlt[:, kd, :],
                            start=(kd == 0), stop=(kd == DCH - 1),
                        )
                    nc.scalar.activation(ft_sb[:, ncI, :], pb[:], AF.Tanh)

                # ---- WcCT_T [t, K] : psum [128, K*TCH], chunk tc -> cols ----
                wcct = work.tile([P, K * TCH], dt)
                ps = psmall.tile([P, K * TCH], dt, tag="psmall")
                for tcI in range(TCH):
                    for kd in range(DCH):
                        nc.tensor.matmul(
                            ps[:, K * tcI:K * (tcI + 1)],
                            ct[:, kd, tcI * P:(tcI + 1) * P],
                            wct_sb[:, kd, :],
                            start=(kd == 0), stop=(kd == DCH - 1),
                        )
                nc.vector.tensor_copy(wcct[:], ps[:])

                # ---- WsST_T [n, K] : psum [128, K*NCH] ----
                wsst = work.tile([P, K * NCH], dt)
                ps = psmall.tile([P, K * NCH], dt, tag="psmall")
                for ncI in range(NCH):
                    for kd in range(DCH):
                        nc.tensor.matmul(
                            ps[:, K * ncI:K * (ncI + 1)],
                            st[:, kd, ncI * P:(ncI + 1) * P],
                            wst_sb[:, kd, :],
                            start=(kd == 0), stop=(kd == DCH - 1),
                        )
                nc.vector.tensor_copy(wsst[:], ps[:])

                # ---- HsT [n, K] = tanh(WsST_T + F^T-contract): G^T[n,k] ----
                ps = psmall.tile([P, K * NCH], dt, tag="psmall")
                for ncI in range(NCH):
                    for tcI in range(TCH):
                        nc.tensor.matmul(
                            ps[:, K * ncI:K * (ncI + 1)],
                            f_sb[:, tcI, ncI * P:(ncI + 1) * P],
                            wcct[:, K * tcI:K * (tcI + 1)],
                            start=(tcI == 0), stop=(tcI == TCH - 1),
                        )
                hst = work.tile([P, K * NCH], dt)
                nc.vector.tensor_add(hst[:], ps[:], wsst[:])
                nc.scalar.activation(hst[:], hst[:], AF.Tanh)

                # ---- HcT [t, K] = tanh(WcCT_T + T2T) ----
                ps = psmall.tile([P, K * TCH], dt, tag="psmall")
                for tcI in range(TCH):
                    for ncI in range(NCH):
                        nc.tensor.matmul(
                            ps[:, K * tcI:K * (tcI + 1)],
                            ft_sb[:, ncI, tcI * P:(tcI + 1) * P],
                            wsst[:, K * ncI:K * (ncI + 1)],
                            start=(ncI == 0), stop=(ncI == NCH - 1),
                        )
                hct = work.tile([P, K * TCH], dt)
                nc.vector.tensor_add(hct[:], ps[:], wcct[:])
                nc.scalar.activation(hct[:], hct[:], AF.Tanh)

                # ---- ES = exp(logits_s) column layout [128, NCH] ----
                hw = work.tile([P, K * NCH], dt)
                nc.vector.tensor_mul(hw[:], hst[:], whs_sb[:])
                es = work.tile([P, NCH], dt)
                nc.vector.tensor_reduce(
                    es[:], hw[:].rearrange("p (a b) -> p a b", b=K),
                    axis=AX.X, op=ALU.add)
                nc.scalar.activation(es[:], es[:], AF.Exp)

                hwc = work.tile([P, K * TCH], dt)
                nc.vector.tensor_mul(hwc[:], hct[:], whc_sb[:])
                ec = work.tile([P, TCH], dt)
                nc.vector.tensor_reduce(
                    ec[:], hwc[:].rearrange("p (a b) -> p a b", b=K),
                    axis=AX.X, op=ALU.add)
                nc.scalar.activation(ec[:], ec[:], AF.Exp)

                # ---- sums over n / t (free reduce, then partition reduce) ----
                rs = work.tile([P, 1], dt, tag="rs")
                nc.vector.tensor_reduce(rs[:], es[:], axis=AX.X, op=ALU.add)
                rc = work.tile([P, 1], dt, tag="rc")
                nc.vector.tensor_reduce(rc[:], ec[:], axis=AX.X, op=ALU.add)
                psum2 = pone.tile([1, 2], dt, tag="pone")
                nc.tensor.matmul(psum2[:, 0:1], rs[:], ones_sb[:],
                                 start=True, stop=True)
                nc.tensor.matmul(psum2[:, 1:2], rc[:], ones_sb[:],
                                 start=True, stop=True)
                rinv = work.tile([1, 2], dt, tag="rinv")
                nc.vector.reciprocal(rinv[:], psum2[:])

                # ---- co_s [1, D] & co_c [1, D] (unnormalized) ----
                pco_s = pone.tile([1, D], dt, tag="pone")
                for ncI in range(NCH):
                    nc.tensor.matmul(
                        pco_s[:], es[:, ncI:ncI + 1], s_nat[:, ncI, :],
                        start=(ncI == 0), stop=(ncI == NCH - 1))
                pco_c = pone.tile([1, D], dt, tag="pone")
                for tcI in range(TCH):
                    nc.tensor.matmul(
                        pco_c[:], ec[:, tcI:tcI + 1], c_nat[:, tcI, :],
                        start=(tcI == 0), stop=(tcI == TCH - 1))

                co_row = work.tile([1, 2 * D], dt, tag="co_row")
                nc.vector.tensor_scalar_mul(co_row[:, 0:D], pco_s[:],
                                            rinv[:, 0:1])
                nc.vector.tensor_scalar_mul(co_row[:, D:2 * D], pco_c[:],
                                            rinv[:, 1:2])

                # ---- transpose co_row -> columns [128, FCH] via K=1 mms ----
                pcol = psmall.tile([P, FCH], dt, tag="psmall")
                for j in range(FCH):
                    nc.tensor.matmul(
                        pcol[:, j:j + 1], co_row[:, j * P:(j + 1) * P],
                        ones_sb[0:1, 0:1], start=True, stop=True)
                ccol = work.tile([P, FCH], dt, tag="ccol")
                nc.vector.tensor_copy(ccol[:], pcol[:])

                # ---- fc: out[1, OUT] ----
                pout = pone.tile([1, OUT], dt, tag="pone")
                for j in range(FCH):
                    nc.tensor.matmul(
                        pout[:], ccol[:, j:j + 1], fcw_sb[:, j, :],
                        start=(j == 0), stop=(j == FCH - 1))
                nc.vector.tensor_add(out_sb[:, b * OUT:(b + 1) * OUT], pout[:], fcb_sb[:])

            nc.sync.dma_start(out_d[:], out_sb[:])

    nc.compile()
    return nc


def _get_nc():
    if "nc" not in _BUILT:
        _BUILT["nc"] = _build_nc()
    return _BUILT["nc"]


def kernel(S, C, Wl, Ws, Wc, Whs, Whc, fc_w, fc_b):
    from concourse.bass_utils import run_bass_kernel_spmd

    S = np.ascontiguousarray(np.asarray(S, dtype=np.float32))
    C = np.ascontiguousarray(np.asarray(C, dtype=np.float32))
    Wl = np.ascontiguousarray(np.asarray(Wl, dtype=np.float32))
    Ws = np.asarray(Ws, dtype=np.float32)
    Wc = np.asarray(Wc, dtype=np.float32)
    Whs = np.asarray(Whs, dtype=np.float32)
    Whc = np.asarray(Whc, dtype=np.float32)
    fc_w = np.asarray(fc_w, dtype=np.float32)
    fc_b = np.asarray(fc_b, dtype=np.float32)

    nc = _get_nc()

    WsT = np.ascontiguousarray(Ws.T)
    WcT = np.ascontiguousarray(Wc.T)
    whs_bc = np.ascontiguousarray(
        np.broadcast_to(np.tile(Whs[0], NCH)[None, :], (P, K * NCH)))
    whc_bc = np.ascontiguousarray(
        np.broadcast_to(np.tile(Whc[0], TCH)[None, :], (P, K * TCH)))
    fcwT = np.ascontiguousarray(fc_w.T)
    fcb = np.ascontiguousarray(fc_b[None, :])
    ones = np.ones((P, 1), dtype=np.float32)

    in_maps = []
    for i in range(N_CORES):
        sl = slice(i * BPC, (i + 1) * BPC)
        in_maps.append({
            "S": np.ascontiguousarray(S[sl]),
            "ST": np.ascontiguousarray(S[sl].transpose(0, 2, 1)),
            "C": np.ascontiguousarray(C[sl]),
            "CT": np.ascontiguousarray(C[sl].transpose(0, 2, 1)),
            "Wl": Wl, "WsT": WsT, "WcT": WcT,
            "whs_bc": whs_bc, "whc_bc": whc_bc,
            "fcwT": fcwT, "fcb": fcb, "ones": ones,
        })

    _BUILT["last_in_maps"] = in_maps
    res = run_bass_kernel_spmd(nc, in_maps, list(range(N_CORES)))
    return np.concatenate(
        [res.results[i]["out"].reshape(BPC, OUT) for i in range(N_CORES)], axis=0)


def __getattr__(name):
    if name == "_LAST_IN_MAPS":
        return _BUILT["last_in_maps"]
    raise AttributeError(name)


# revision 4
# speedup vs baseline: 2.8665x; 2.8665x over previous
"""CoAttentionNetwork Trainium2 kernel — 8-core data parallel over batch.

Takes FULL inputs (B=64), shards batch across 8 NeuronCores (8 batches per
core), runs a Bass/Tile kernel per core, gathers per-core outputs.

Per-batch device algorithm (b = one of 8 local batches):
  CWlT[D',t] = Wl^T C^T           (bf16 matmuls, fp32 psum)
  F[t,n]    = tanh(CWlT^T S^T)    -> bf16
  FT[n,t]   = tanh(S CWlT)        -> bf16 (recomputed, not transposed)
  Hs[2,n]   = tanh(Ws S^T + (Wc C^T) F)     row layout, 2-col stationary
  Hc[2,t]   = tanh(Wc C^T + (Ws S^T) F^T)   row layout
  logits -> exp (no max-sub: |logit| < 2) -> column layout [128, chunks]
  co_s = S^T exp_s / sum, co_c = C^T exp_c / sum   (fp32)
  out[b] = fc_w @ [co_s; co_c] + fc_b              (fp32)
"""

import numpy as np

B, N, T, D, K, OUT = 64, 1024, 512, 384, 2, 6
N_CORES = 8
BPC = B // N_CORES  # batches per core
P = 128
NCH = N // P   # 8 n-chunks
TCH = T // P   # 4 t-chunks
DCH = D // P   # 3 d-chunks
FCH = 2 * D // P  # 6 chunks of concat dim

_BUILT = {}


def _build_nc():
    import concourse.bacc as bacc
    import concourse.mybir as mybir
    import concourse.tile as tile

    f32 = mybir.dt.float32
    bf16 = mybir.dt.bfloat16
    AF = mybir.ActivationFunctionType
    AX = mybir.AxisListType
    ALU = mybir.AluOpType

    nc = bacc.Bacc(None, target_bir_lowering=False, debug=False)

    S_d = nc.dram_tensor("S", [BPC, N, D], f32, kind="ExternalInput")
    ST_d = nc.dram_tensor("ST", [BPC, D, N], bf16, kind="ExternalInput")
    C_d = nc.dram_tensor("C", [BPC, T, D], f32, kind="ExternalInput")
    CT_d = nc.dram_tensor("CT", [BPC, D, T], bf16, kind="ExternalInput")
    Wl_d = nc.dram_tensor("Wl", [D, D], bf16, kind="ExternalInput")
    WsT_d = nc.dram_tensor("WsT", [D, K], bf16, kind="ExternalInput")
    WcT_d = nc.dram_tensor("WcT", [D, K], bf16, kind="ExternalInput")
    whsT_d = nc.dram_tensor("whsT", [K, 1], f32, kind="ExternalInput")
    whcT_d = nc.dram_tensor("whcT", [K, 1], f32, kind="ExternalInput")
    fcwT_d = nc.dram_tensor("fcwT", [2 * D, OUT], f32, kind="ExternalInput")
    fcb_d = nc.dram_tensor("fcb", [1, OUT], f32, kind="ExternalInput")
    ones_d = nc.dram_tensor("ones", [P, 1], f32, kind="ExternalInput")
    out_d = nc.dram_tensor("out", [1, BPC * OUT], f32, kind="ExternalOutput")

    with tile.TileContext(nc) as tc:
        with (
            tc.tile_pool(name="wpool", bufs=1) as wpool,
            tc.tile_pool(name="io", bufs=2) as io,
            tc.tile_pool(name="work", bufs=2) as work,
            tc.tile_pool(name="pbig", bufs=4, space="PSUM") as pbig,
            tc.tile_pool(name="psmall", bufs=2, space="PSUM") as psmall,
            tc.tile_pool(name="pone", bufs=2, space="PSUM") as pone,
        ):
            # ---- constants / weights (loaded once) ----
            wl_sb = wpool.tile([P, DCH, D], bf16)
            nc.sync.dma_start(wl_sb[:], Wl_d.rearrange("(c p) m -> p c m", p=P))
            wst_sb = wpool.tile([P, DCH, K], bf16)
            nc.sync.dma_start(wst_sb[:], WsT_d.rearrange("(c p) k -> p c k", p=P))
            wct_sb = wpool.tile([P, DCH, K], bf16)
            nc.sync.dma_start(wct_sb[:], WcT_d.rearrange("(c p) k -> p c k", p=P))
            whst_sb = wpool.tile([K, 1], f32)
            nc.sync.dma_start(whst_sb[:], whsT_d[:])
            whct_sb = wpool.tile([K, 1], f32)
            nc.sync.dma_start(whct_sb[:], whcT_d[:])
            fcw_sb = wpool.tile([P, FCH, OUT], f32)
            nc.sync.dma_start(fcw_sb[:], fcwT_d.rearrange("(c p) o -> p c o", p=P))
            fcb_sb = wpool.tile([1, OUT], f32)
            nc.sync.dma_start(fcb_sb[:], fcb_d[:])
            ones_sb = wpool.tile([P, 1], f32)
            nc.sync.dma_start(ones_sb[:], ones_d[:])
            out_sb = wpool.tile([1, BPC * OUT], f32)

            for b in range(BPC):
                # ---- input DMAs for this batch ----
                s_nat = io.tile([P, NCH, D], f32)
                nc.sync.dma_start(s_nat[:], S_d[b].rearrange("(c p) d -> p c d", p=P))
                st = io.tile([P, DCH, N], bf16)
                nc.sync.dma_start(st[:], ST_d[b].rearrange("(c p) n -> p c n", p=P))
                c_nat = io.tile([P, TCH, D], f32)
                nc.sync.dma_start(c_nat[:], C_d[b].rearrange("(c p) d -> p c d", p=P))
                ct = io.tile([P, DCH, T], bf16)
                nc.sync.dma_start(ct[:], CT_d[b].rearrange("(c p) t -> p c t", p=P))

                # ---- CWlT [D', t] = Wl^T @ C^T : 3 chunks of [128, 512] ----
                cwlt = work.tile([P, DCH, T], bf16)
                for dc in range(DCH):
                    pb = pbig.tile([P, 512], f32, tag="pbig")
                    for kd in range(DCH):
                        nc.tensor.matmul(
                            pb[:],
                            wl_sb[:, kd, dc * P:(dc + 1) * P],
                            ct[:, kd, :],
                            start=(kd == 0), stop=(kd == DCH - 1),
                        )
                    nc.vector.tensor_copy(cwlt[:, dc, :], pb[:])

                # ---- F [t, n] = tanh(CWlT^T @ ST): 4 t-chunks x 2 n-halves ----
                f_sb = work.tile([P, TCH, N], bf16)
                for tcI in range(TCH):
                    for nh in range(2):
                        pb = pbig.tile([P, 512], f32, tag="pbig")
                        for kd in range(DCH):
                            nc.tensor.matmul(
                                pb[:],
                                cwlt[:, kd, tcI * P:(tcI + 1) * P],
                                st[:, kd, nh * 512:(nh + 1) * 512],
                                start=(kd == 0), stop=(kd == DCH - 1),
                            )
                        nc.scalar.activation(
                            f_sb[:, tcI, nh * 512:(nh + 1) * 512], pb[:], AF.Tanh)

                # ---- FT [n, t] = tanh(ST^T @ CWlT): 8 n-chunks ----
                ft_sb = work.tile([P, NCH, T], bf16)
                for ncI in range(NCH):
                    pb = pbig.tile([P, 512], f32, tag="pbig")
                    for kd in range(DCH):
                        nc.tensor.matmul(
                            pb[:],
                            st[:, kd, ncI * P:(ncI + 1) * P],
                            cwlt[:, kd, :],
                            start=(kd == 0), stop=(kd == DCH - 1),
                        )
                    nc.scalar.activation(ft_sb[:, ncI, :], pb[:], AF.Tanh)

                # ---- WcCT_T [t, K] column layout (rhs/lhsT for G, Hc) ----
                wcct = work.tile([P, K * TCH], bf16)
                ps = psmall.tile([P, K * TCH], f32, tag="psmall")
                for tcI in range(TCH):
                    for kd in range(DCH):
                        nc.tensor.matmul(
                            ps[:, K * tcI:K * (tcI + 1)],
                            ct[:, kd, tcI * P:(tcI + 1) * P],
                            wct_sb[:, kd, :],
                            start=(kd == 0), stop=(kd == DCH - 1),
                        )
                nc.vector.tensor_copy(wcct[:], ps[:])

                # ---- WsST_T [n, K] column layout (lhsT for T2 in Hc) ----
                wsst = work.tile([P, K * NCH], bf16)
                ps = psmall.tile([P, K * NCH], f32, tag="psmall")
                for ncI in range(NCH):
                    for kd in range(DCH):
                        nc.tensor.matmul(
                            ps[:, K * ncI:K * (ncI + 1)],
                            st[:, kd, ncI * P:(ncI + 1) * P],
                            wst_sb[:, kd, :],
                            start=(kd == 0), stop=(kd == DCH - 1),
                        )
                nc.vector.tensor_copy(wsst[:], ps[:])

                # ---- Hs row [2, N] = tanh(Ws S^T + WcC F) ----
                hs_row = work.tile([K, N], f32, tag="hs_row")
                for nh in range(2):
                    ph = pone.tile([K, 512], f32, tag="pone")
                    for kd in range(DCH):
                        nc.tensor.matmul(
                            ph[:], wst_sb[:, kd, :],
                            st[:, kd, nh * 512:(nh + 1) * 512],
                            start=(kd == 0), stop=False)
                    for tcI in range(TCH):
                        nc.tensor.matmul(
                            ph[:], wcct[:, K * tcI:K * (tcI + 1)],
                            f_sb[:, tcI, nh * 512:(nh + 1) * 512],
                            start=False, stop=(tcI == TCH - 1))
                    nc.scalar.activation(hs_row[:, nh * 512:(nh + 1) * 512],
                                         ph[:], AF.Tanh)

                # ---- Hc row [2, T] = tanh(Wc C^T + WsS F^T) ----
                hc_row = work.tile([K, T], f32, tag="hc_row")
                ph = pone.tile([K, 512], f32, tag="pone")
                for kd in range(DCH):
                    nc.tensor.matmul(
                        ph[:], wct_sb[:, kd, :], ct[:, kd, :],
                        start=(kd == 0), stop=False)
                for ncI in range(NCH):
                    nc.tensor.matmul(
                        ph[:], wsst[:, K * ncI:K * (ncI + 1)],
                        ft_sb[:, ncI, :],
                        start=False, stop=(ncI == NCH - 1))
                nc.scalar.activation(hc_row[:], ph[:], AF.Tanh)

                # ---- logits -> exp, column layout ----
                ps = psmall.tile([P, NCH], f32, tag="psmall")
                for ncI in range(NCH):
                    nc.tensor.matmul(
                        ps[:, ncI:ncI + 1],
                        hs_row[:, ncI * P:(ncI + 1) * P], whst_sb[:],
                        start=True, stop=True)
                es = work.tile([P, NCH], f32, tag="es")
                nc.scalar.activation(es[:], ps[:], AF.Exp)

                ps = psmall.tile([P, TCH], f32, tag="psmall")
                for tcI in range(TCH):
                    nc.tensor.matmul(
                        ps[:, tcI:tcI + 1],
                        hc_row[:, tcI * P:(tcI + 1) * P], whct_sb[:],
                        start=True, stop=True)
                ec = work.tile([P, TCH], f32, tag="ec")
                nc.scalar.activation(ec[:], ps[:], AF.Exp)

                # ---- sums over n / t (free reduce, then partition reduce) ----
                rs = work.tile([P, 1], f32, tag="rs")
                nc.vector.tensor_reduce(rs[:], es[:], axis=AX.X, op=ALU.add)
                rc = work.tile([P, 1], f32, tag="rc")
                nc.vector.tensor_reduce(rc[:], ec[:], axis=AX.X, op=ALU.add)
                psum2 = pone.tile([1, 2], f32, tag="pone")
                nc.tensor.matmul(psum2[:, 0:1], rs[:], ones_sb[:],
                                 start=True, stop=True)
                nc.tensor.matmul(psum2[:, 1:2], rc[:], ones_sb[:],
                                 start=True, stop=True)
                rinv = work.tile([1, 2], f32, tag="rinv")
                nc.vector.reciprocal(rinv[:], psum2[:])

                # ---- co_s [1, D] & co_c [1, D] (unnormalized) ----
                pco_s = pone.tile([1, D], f32, tag="pone")
                for ncI in range(NCH):
                    nc.tensor.matmul(
                        pco_s[:], es[:, ncI:ncI + 1], s_nat[:, ncI, :],
                        start=(ncI == 0), stop=(ncI == NCH - 1))
                pco_c = pone.tile([1, D], f32, tag="pone")
                for tcI in range(TCH):
                    nc.tensor.matmul(
                        pco_c[:], ec[:, tcI:tcI + 1], c_nat[:, tcI, :],
                        start=(tcI == 0), stop=(tcI == TCH - 1))

                co_row = work.tile([1, 2 * D], f32, tag="co_row")
                nc.vector.tensor_scalar_mul(co_row[:, 0:D], pco_s[:],
                                            rinv[:, 0:1])
                nc.vector.tensor_scalar_mul(co_row[:, D:2 * D], pco_c[:],
                                            rinv[:, 1:2])

                # ---- transpose co_row -> columns [128, FCH] via K=1 mms ----
                pcol = psmall.tile([P, FCH], f32, tag="psmall")
                for j in range(FCH):
                    nc.tensor.matmul(
                        pcol[:, j:j + 1], co_row[:, j * P:(j + 1) * P],
                        ones_sb[0:1, 0:1], start=True, stop=True)
                ccol = work.tile([P, FCH], f32, tag="ccol")
                nc.vector.tensor_copy(ccol[:], pcol[:])

                # ---- fc: out[1, OUT] ----
                pout = pone.tile([1, OUT], f32, tag="pone")
                for j in range(FCH):
                    nc.tensor.matmul(
                        pout[:], ccol[:, j:j + 1], fcw_sb[:, j, :],
                        start=(j == 0), stop=(j == FCH - 1))
                nc.vector.tensor_add(out_sb[:, b * OUT:(b + 1) * OUT],
                                     pout[:], fcb_sb[:])

            nc.sync.dma_start(out_d[:], out_sb[:])

    nc.compile()
    return nc


def _get_nc():
    if "nc" not in _BUILT:
        _BUILT["nc"] = _build_nc()
    return _BUILT["nc"]


def kernel(S, C, Wl, Ws, Wc, Whs, Whc, fc_w, fc_b):
    import ml_dtypes
    from concourse.bass_utils import run_bass_kernel_spmd

    bf = ml_dtypes.bfloat16
    S = np.ascontiguousarray(np.asarray(S, dtype=np.float32))
    C = np.ascontiguousarray(np.asarray(C, dtype=np.float32))
    Wl = np.asarray(Wl, dtype=np.float32)
    Ws = np.asarray(Ws, dtype=np.float32)
    Wc = np.asarray(Wc, dtype=np.float32)
    Whs = np.asarray(Whs, dtype=np.float32)
    Whc = np.asarray(Whc, dtype=np.float32)
    fc_w = np.asarray(fc_w, dtype=np.float32)
    fc_b = np.asarray(fc_b, dtype=np.float32)

    nc = _get_nc()

    WsT = np.ascontiguousarray(Ws.T.astype(bf))
    WcT = np.ascontiguousarray(Wc.T.astype(bf))
    Wlb = np.ascontiguousarray(Wl.astype(bf))
    whsT = np.ascontiguousarray(Whs.T)          # [K, 1] f32
    whcT = np.ascontiguousarray(Whc.T)
    fcwT = np.ascontiguousarray(fc_w.T)
    fcb = np.ascontiguousarray(fc_b[None, :])
    ones = np.ones((P, 1), dtype=np.float32)

    in_maps = []
    for i in range(N_CORES):
        sl = slice(i * BPC, (i + 1) * BPC)
        in_maps.append({
            "S": np.ascontiguousarray(S[sl]),
            "ST": np.ascontiguousarray(S[sl].transpose(0, 2, 1).astype(bf)),
            "C": np.ascontiguousarray(C[sl]),
            "CT": np.ascontiguousarray(C[sl].transpose(0, 2, 1).astype(bf)),
            "Wl": Wlb, "WsT": WsT, "WcT": WcT,
            "whsT": whsT, "whcT": whcT,
            "fcwT": fcwT, "fcb": fcb, "ones": ones,
        })

    _BUILT["last_in_maps"] = in_maps
    res = run_bass_kernel_spmd(nc, in_maps, list(range(N_CORES)))
    return np.concatenate(
        [res.results[i]["out"].reshape(BPC, OUT) for i in range(N_CORES)], axis=0)


def __getattr__(name):
    if name == "_LAST_IN_MAPS":
        return _BUILT["last_in_maps"]
    raise AttributeError(name)


# revision 6
# speedup vs baseline: 3.5701x; 1.2454x over previous
"""CoAttentionNetwork Trainium2 kernel — 8-core data parallel over batch.

Takes FULL inputs (B=64), shards batch across 8 NeuronCores (8 batches per
core), runs a Bass/Tile kernel per core, gathers per-core outputs.

Per-batch device algorithm (b = one of 8 local batches):
  CWlT[D',t] = Wl^T C^T           (bf16 matmuls, fp32 psum)
  F[t,n]    = tanh(CWlT^T S^T)    -> bf16
  FT[n,t]   = tanh(S CWlT)        -> bf16 (recomputed, not transposed)
  WcC row [2,T] -> snapshot -> transpose-mm -> column form for G
  Hs[2,n]   = tanh(Ws S^T + (Wc C^T) F)     row layout, 2-col stationary;
              WsS snapshot mid-accumulation -> transpose-mm -> cols for T2
  Hc[2,t]   = tanh(Wc C^T + (Ws S^T) F^T)   row layout
  logits -> exp (no max-sub: |logit| < 2) -> column layout [128, chunks]
  co_s = S^T exp_s / sum, co_c = C^T exp_c / sum
  out[b] = fc_w @ [co_s; co_c] + fc_b
"""

import numpy as np

B, N, T, D, K, OUT = 64, 1024, 512, 384, 2, 6
N_CORES = 8
BPC = B // N_CORES  # batches per core
P = 128
NCH = N // P   # 8 n-chunks
TCH = T // P   # 4 t-chunks
DCH = D // P   # 3 d-chunks
FCH = 2 * D // P  # 6 chunks of concat dim

_BUILT = {}


def _build_nc():
    import concourse.bacc as bacc
    import concourse.mybir as mybir
    import concourse.tile as tile

    f32 = mybir.dt.float32
    bf16 = mybir.dt.bfloat16
    AF = mybir.ActivationFunctionType
    AX = mybir.AxisListType
    ALU = mybir.AluOpType

    nc = bacc.Bacc(None, target_bir_lowering=False, debug=False)

    S_d = nc.dram_tensor("S", [BPC, N, D], bf16, kind="ExternalInput")
    ST_d = nc.dram_tensor("ST", [BPC, D, N], bf16, kind="ExternalInput")
    C_d = nc.dram_tensor("C", [BPC, T, D], bf16, kind="ExternalInput")
    CT_d = nc.dram_tensor("CT", [BPC, D, T], bf16, kind="ExternalInput")
    Wl_d = nc.dram_tensor("Wl", [D, D], bf16, kind="ExternalInput")
    WsT_d = nc.dram_tensor("WsT", [D, K], bf16, kind="ExternalInput")
    WcT_d = nc.dram_tensor("WcT", [D, K], bf16, kind="ExternalInput")
    whsT_d = nc.dram_tensor("whsT", [K, 1], f32, kind="ExternalInput")
    whcT_d = nc.dram_tensor("whcT", [K, 1], f32, kind="ExternalInput")
    eye2_d = nc.dram_tensor("eye2", [K, K], bf16, kind="ExternalInput")
    fcwT_d = nc.dram_tensor("fcwT", [2 * D, OUT], bf16, kind="ExternalInput")
    fcb_d = nc.dram_tensor("fcb", [1, OUT], f32, kind="ExternalInput")
    ones_d = nc.dram_tensor("ones", [P, 1], f32, kind="ExternalInput")
    onesb_d = nc.dram_tensor("onesb", [1, 1], bf16, kind="ExternalInput")
    out_d = nc.dram_tensor("out", [1, BPC * OUT], f32, kind="ExternalOutput")

    with tile.TileContext(nc) as tc:
        with (
            tc.tile_pool(name="wpool", bufs=1) as wpool,
            tc.tile_pool(name="io", bufs=2) as io,
            tc.tile_pool(name="work", bufs=2) as work,
            tc.tile_pool(name="pbig", bufs=3, space="PSUM") as pbig,
            tc.tile_pool(name="psmall", bufs=2, space="PSUM") as psmall,
            tc.tile_pool(name="pph", bufs=2, space="PSUM") as pph,
            tc.tile_pool(name="pphc", bufs=1, space="PSUM") as pphc,
        ):
            # ---- constants / weights (loaded once) ----
            wl_sb = wpool.tile([P, DCH, D], bf16)
            nc.sync.dma_start(wl_sb[:], Wl_d.rearrange("(c p) m -> p c m", p=P))
            wst_sb = wpool.tile([P, DCH, K], bf16)
            nc.sync.dma_start(wst_sb[:], WsT_d.rearrange("(c p) k -> p c k", p=P))
            wct_sb = wpool.tile([P, DCH, K], bf16)
            nc.sync.dma_start(wct_sb[:], WcT_d.rearrange("(c p) k -> p c k", p=P))
            whst_sb = wpool.tile([K, 1], f32)
            nc.sync.dma_start(whst_sb[:], whsT_d[:])
            whct_sb = wpool.tile([K, 1], f32)
            nc.sync.dma_start(whct_sb[:], whcT_d[:])
            eye2_sb = wpool.tile([K, K], bf16)
            nc.sync.dma_start(eye2_sb[:], eye2_d[:])
            fcw_sb = wpool.tile([P, FCH, OUT], bf16)
            nc.sync.dma_start(fcw_sb[:], fcwT_d.rearrange("(c p) o -> p c o", p=P))
            fcb_sb = wpool.tile([1, OUT], f32)
            nc.sync.dma_start(fcb_sb[:], fcb_d[:])
            ones_sb = wpool.tile([P, 1], f32)
            nc.sync.dma_start(ones_sb[:], ones_d[:])
            onesb_sb = wpool.tile([1, 1], bf16)
            nc.sync.dma_start(onesb_sb[:], onesb_d[:])
            out_sb = wpool.tile([1, BPC * OUT], f32)

            for b in range(BPC):
                # ---- input DMAs for this batch ----
                s_nat = io.tile([P, NCH, D], bf16)
                nc.sync.dma_start(s_nat[:], S_d[b].rearrange("(c p) d -> p c d", p=P))
                st = io.tile([P, DCH, N], bf16)
                nc.sync.dma_start(st[:], ST_d[b].rearrange("(c p) n -> p c n", p=P))
                c_nat = io.tile([P, TCH, D], bf16)
                nc.sync.dma_start(c_nat[:], C_d[b].rearrange("(c p) d -> p c d", p=P))
                ct = io.tile([P, DCH, T], bf16)
                nc.sync.dma_start(ct[:], CT_d[b].rearrange("(c p) t -> p c t", p=P))

                # ---- CWlT [D', t] = Wl^T @ C^T : 3 chunks of [128, 512] ----
                cwlt = work.tile([P, DCH, T], bf16)
                for dc in range(DCH):
                    pb = pbig.tile([P, 512], f32, tag="pbig")
                    for kd in range(DCH):
                        nc.tensor.matmul(
                            pb[:],
                            wl_sb[:, kd, dc * P:(dc + 1) * P],
                            ct[:, kd, :],
                            start=(kd == 0), stop=(kd == DCH - 1),
                        )
                    nc.vector.tensor_copy(cwlt[:, dc, :], pb[:])

                # ---- F [t, n] = tanh(CWlT^T @ ST): 4 t-chunks x 2 halves ----
                f_sb = work.tile([P, TCH, N], bf16)
                for tcI in range(TCH):
                    for nh in range(2):
                        pb = pbig.tile([P, 512], f32, tag="pbig")
                        for kd in range(DCH):
                            nc.tensor.matmul(
                                pb[:],
                                cwlt[:, kd, tcI * P:(tcI + 1) * P],
                                st[:, kd, nh * 512:(nh + 1) * 512],
                                start=(kd == 0), stop=(kd == DCH - 1),
                            )
                        nc.scalar.activation(
                            f_sb[:, tcI, nh * 512:(nh + 1) * 512], pb[:], AF.Tanh)

                # ---- FT [n, t] = tanh(ST^T @ CWlT): 8 n-chunks ----
                ft_sb = work.tile([P, NCH, T], bf16)
                for ncI in range(NCH):
                    pb = pbig.tile([P, 512], f32, tag="pbig")
                    for kd in range(DCH):
                        nc.tensor.matmul(
                            pb[:],
                            st[:, kd, ncI * P:(ncI + 1) * P],
                            cwlt[:, kd, :],
                            start=(kd == 0), stop=(kd == DCH - 1),
                        )
                    nc.scalar.activation(ft_sb[:, ncI, :], pb[:], AF.Tanh)

                # ---- Hc part 1: WcC row [2, T]; snapshot for G's lhsT ----
                ph_c = pphc.tile([K, T], f32, tag="ph_c")
                for kd in range(DCH):
                    nc.tensor.matmul(
                        ph_c[:], wct_sb[:, kd, :], ct[:, kd, :],
                        start=(kd == 0), stop=False)
                wcc_row = work.tile([K, T], bf16, tag="wcc_row")
                nc.vector.tensor_copy(wcc_row[:], ph_c[:])
                # transpose to column form [128, K*TCH]
                ps = psmall.tile([P, K * TCH], f32, tag="psmall")
                for tcI in range(TCH):
                    nc.tensor.matmul(
                        ps[:, K * tcI:K * (tcI + 1)],
                        wcc_row[:, tcI * P:(tcI + 1) * P], eye2_sb[:],
                        start=True, stop=True)
                wcct = work.tile([P, K * TCH], bf16)
                nc.vector.tensor_copy(wcct[:], ps[:])

                # ---- Hs row [2, N] = tanh(WsS + G); WsS snapshot mid-way ----
                wss_row = work.tile([K, N], bf16, tag="wss_row")
                hs_row = work.tile([K, N], f32, tag="hs_row")
                for nh in range(2):
                    sl = slice(nh * 512, (nh + 1) * 512)
                    ph = pph.tile([K, 512], f32, tag="ph")
                    for kd in range(DCH):
                        nc.tensor.matmul(
                            ph[:], wst_sb[:, kd, :], st[:, kd, sl],
                            start=(kd == 0), stop=False)
                    nc.vector.tensor_copy(wss_row[:, sl], ph[:])
                    for tcI in range(TCH):
                        nc.tensor.matmul(
                            ph[:], wcct[:, K * tcI:K * (tcI + 1)],
                            f_sb[:, tcI, sl],
                            start=False, stop=(tcI == TCH - 1))
                    nc.scalar.activation(hs_row[:, sl], ph[:], AF.Tanh)

                # WsS^T column form [128, K*NCH] for T2's lhsT
                ps = psmall.tile([P, K * NCH], f32, tag="psmall")
                for ncI in range(NCH):
                    nc.tensor.matmul(
                        ps[:, K * ncI:K * (ncI + 1)],
                        wss_row[:, ncI * P:(ncI + 1) * P], eye2_sb[:],
                        start=True, stop=True)
                wsst = work.tile([P, K * NCH], bf16)
                nc.vector.tensor_copy(wsst[:], ps[:])

                # ---- Hc part 2: T2 accumulate onto WcC, tanh ----
                for ncI in range(NCH):
                    nc.tensor.matmul(
                        ph_c[:], wsst[:, K * ncI:K * (ncI + 1)],
                        ft_sb[:, ncI, :],
                        start=False, stop=(ncI == NCH - 1))
                hc_row = work.tile([K, T], f32, tag="hc_row")
                nc.scalar.activation(hc_row[:], ph_c[:], AF.Tanh)

                # ---- logits -> exp, column layout ----
                ps = psmall.tile([P, NCH], f32, tag="psmall")
                for ncI in range(NCH):
                    nc.tensor.matmul(
                        ps[:, ncI:ncI + 1],
                        hs_row[:, ncI * P:(ncI + 1) * P], whst_sb[:],
                        start=True, stop=True)
                es = work.tile([P, NCH], bf16, tag="es")
                nc.scalar.activation(es[:], ps[:], AF.Exp)
                esf = work.tile([P, NCH], f32, tag="esf")
                nc.vector.tensor_copy(esf[:], es[:])

                ps = psmall.tile([P, TCH], f32, tag="psmall")
                for tcI in range(TCH):
                    nc.tensor.matmul(
                        ps[:, tcI:tcI + 1],
                        hc_row[:, tcI * P:(tcI + 1) * P], whct_sb[:],
                        start=True, stop=True)
                ec = work.tile([P, TCH], bf16, tag="ec")
                nc.scalar.activation(ec[:], ps[:], AF.Exp)
                ecf = work.tile([P, TCH], f32, tag="ecf")
                nc.vector.tensor_copy(ecf[:], ec[:])

                # ---- sums over n / t (free reduce, then partition reduce) ----
                rs = work.tile([P, 1], f32, tag="rs")
                nc.vector.tensor_reduce(rs[:], esf[:], axis=AX.X, op=ALU.add)
                rc = work.tile([P, 1], f32, tag="rc")
                nc.vector.tensor_reduce(rc[:], ecf[:], axis=AX.X, op=ALU.add)
                psum2 = psmall.tile([1, 2], f32, tag="psmall")
                nc.tensor.matmul(psum2[:, 0:1], rs[:], ones_sb[:],
                                 start=True, stop=True)
                nc.tensor.matmul(psum2[:, 1:2], rc[:], ones_sb[:],
                                 start=True, stop=True)
                rinv = work.tile([1, 2], f32, tag="rinv")
                nc.vector.reciprocal(rinv[:], psum2[:])

                # ---- co_s [1, D] & co_c [1, D] (unnormalized, bf16 mms) ----
                pco_s = psmall.tile([1, D], f32, tag="psmall")
                for ncI in range(NCH):
                    nc.tensor.matmul(
                        pco_s[:], es[:, ncI:ncI + 1], s_nat[:, ncI, :],
                        start=(ncI == 0), stop=(ncI == NCH - 1))
                pco_c = psmall.tile([1, D], f32, tag="psmall")
                for tcI in range(TCH):
                    nc.tensor.matmul(
                        pco_c[:], ec[:, tcI:tcI + 1], c_nat[:, tcI, :],
                        start=(tcI == 0), stop=(tcI == TCH - 1))

                co_row = work.tile([1, 2 * D], bf16, tag="co_row")
                nc.vector.tensor_scalar_mul(co_row[:, 0:D], pco_s[:],
                                            rinv[:, 0:1])
                nc.vector.tensor_scalar_mul(co_row[:, D:2 * D], pco_c[:],
                                            rinv[:, 1:2])

                # ---- transpose co_row -> columns [128, FCH] via K=1 mms ----
                pcol = psmall.tile([P, FCH], f32, tag="psmall")
                for j in range(FCH):
                    nc.tensor.matmul(
                        pcol[:, j:j + 1], co_row[:, j * P:(j + 1) * P],
                        onesb_sb[:], start=True, stop=True)
                ccol = work.tile([P, FCH], bf16, tag="ccol")
                nc.vector.tensor_copy(ccol[:], pcol[:])

                # ---- fc: out[1, OUT] ----
                pout = psmall.tile([1, OUT], f32, tag="psmall")
                for j in range(FCH):
                    nc.tensor.matmul(
                        pout[:], ccol[:, j:j + 1], fcw_sb[:, j, :],
                        start=(j == 0), stop=(j == FCH - 1))
                nc.vector.tensor_add(out_sb[:, b * OUT:(b + 1) * OUT],
                                     pout[:], fcb_sb[:])

            nc.sync.dma_start(out_d[:], out_sb[:])

    nc.compile()
    return nc


def _get_nc():
    if "nc" not in _BUILT:
        _BUILT["nc"] = _build_nc()
    return _BUILT["nc"]


def kernel(S, C, Wl, Ws, Wc, Whs, Whc, fc_w, fc_b):
    import ml_dtypes
    from concourse.bass_utils import run_bass_kernel_spmd

    bf = ml_dtypes.bfloat16
    S = np.ascontiguousarray(np.asarray(S, dtype=np.float32))
    C = np.ascontiguousarray(np.asarray(C, dtype=np.float32))
    Wl = np.asarray(Wl, dtype=np.float32)
    Ws = np.asarray(Ws, dtype=np.float32)
    Wc = np.asarray(Wc, dtype=np.float32)
    Whs = np.asarray(Whs, dtype=np.float32)
    Whc = np.asarray(Whc, dtype=np.float32)
    fc_w = np.asarray(fc_w, dtype=np.float32)
    fc_b = np.asarray(fc_b, dtype=np.float32)

    nc = _get_nc()

    in_common = {
        "Wl": np.ascontiguousarray(Wl.astype(bf)),
        "WsT": np.ascontiguousarray(Ws.T.astype(bf)),
        "WcT": np.ascontiguousarray(Wc.T.astype(bf)),
        "whsT": np.ascontiguousarray(Whs.T),
        "whcT": np.ascontiguousarray(Whc.T),
        "eye2": np.eye(K, dtype=bf),
        "fcwT": np.ascontiguousarray(fc_w.T.astype(bf)),
        "fcb": np.ascontiguousarray(fc_b[None, :]),
        "ones": np.ones((P, 1), dtype=np.float32),
        "onesb": np.ones((1, 1), dtype=bf),
    }
    in_maps = []
    for i in range(N_CORES):
        sl = slice(i * BPC, (i + 1) * BPC)
        in_maps.append({
            "S": np.ascontiguousarray(S[sl].astype(bf)),
            "ST": np.ascontiguousarray(S[sl].transpose(0, 2, 1).astype(bf)),
            "C": np.ascontiguousarray(C[sl].astype(bf)),
            "CT": np.ascontiguousarray(C[sl].transpose(0, 2, 1).astype(bf)),
            **in_common,
        })

    _BUILT["last_in_maps"] = in_maps
    res = run_bass_kernel_spmd(nc, in_maps, list(range(N_CORES)))
    return np.concatenate(
        [res.results[i]["out"].reshape(BPC, OUT) for i in range(N_CORES)], axis=0)


def __getattr__(name):
    if name == "_LAST_IN_MAPS":
        return _BUILT["last_in_maps"]
    raise AttributeError(name)
